# revision 12
# baseline (speedup 1.0000x reference)
"""MemSAC loss (retrieval kNN + masked log-softmax contrastive loss) on 8
Trainium2 cores.

Sharding: the 48000-slot memory queue is split 6000 rows/core (no
padding). The host substitutes the enqueued source rows, L2-normalizes,
and ships per core the transposed queue shard (for the similarity
matmuls), the per-column labels as i32 (broadcast into the low u16
halves of the packed top-k buffers), the GLOBAL per-class feature sums
W = sum_{label==c} qn_row (a label-histogram-weighted sum, same O(Q*D)
host glue as the normalization) and the global per-class counts. With W
global on every core, S = tgtn @ W is a local matmul and the W
AllReduce of the previous design disappears — the only collective left
is the candidate AllGather.

Device pipeline per core (chunk-major, 3 superblocks of 2000 cols):
  - S = tgtT^T @ W (8 small matmuls into one PSUM bank, one ACT copy)
    runs at load time, entirely off the critical path
  - per slot (target chunk c, superblock s): PE matmul [128x2000] into
    PSUM; one ACT pass computes exp(sim/tau) writing bf16 into the high
    u16 halves of an i32 buffer whose low halves hold the column LABELS
    (DMA-broadcast once per superblock); as f32 bit patterns these are
    order-isomorphic to sim with label tie-break, so one DVE max8 per
    slot yields top-8 (value,label) pairs atomically; ACT accum_out
    accumulates the sum-exp denominator per slot
  - pack: per chunk max8-of-24 -> top-5 packed (value|label) + sum-exp
    -> [512 x 6] f32; ONE AllGather (12KB in / 98KB out); a junk Ln op
    preloads the ACT Ln table under the gather
  - every core redundantly merges the 8x5 candidates -> global top-5 ->
    majority vote (all-pairs equality) -> pseudo label -> S[t,pseudo] /
    cnt[pseudo] and log(sum-exp) -> per-sample losses -> DMA out; the
    host applies the warm-up coefficient and the batch mean.

PSUM accumulation groups are kept strictly contiguous on the PE via an
explicit instruction chain (the PE faults if two groups interleave).

kernel() takes FULL unsharded inputs and returns the FULL scalar output.
"""

import os
import sys

sys.path.insert(0, "/opt/trn_rl_repo")
os.environ.setdefault("MYCRO_LOCAL_CACHE", "1")

import numpy as np
from contextlib import ExitStack

import concourse.bass as bass
import concourse.bacc as bacc
import concourse.tile as tile
from concourse.tile import add_dep_helper
from concourse import mybir
from concourse.bass_utils import run_bass_kernel_spmd

AF = mybir.ActivationFunctionType
AL = mybir.AluOpType
AX = mybir.AxisListType
F32 = mybir.dt.float32
BF16 = mybir.dt.bfloat16
I32 = mybir.dt.int32
U16 = mybir.dt.uint16
NP_BF16 = mybir.dt.np(mybir.dt.bfloat16)

SKIP_GC = False
# ---- problem constants ----
D = 256
Q = 48000
C = 126          # n classes
BS = 512         # source batch
BT = 512         # target batch
TAU = 0.07
COEFF = 0.1
WARM_UP = 4000
NCORES = 8
QS = Q // NCORES            # 6000 rows per core, no padding
QSP = QS                    # kept for test.py compatibility
K = 5                       # neighbors kept
SB = 2000                   # superblock width (4 PSUM banks)
NSB = QS // SB              # 3 superblocks


def build_program(n_cores, qsp=QSP, bt=BT, stage=8, mock_cc=False,
                  n_reps=1, chain=False):
    """Build the SPMD Bass program. Identical NEFF runs on all cores."""
    tcn = bt // 128             # 4 target chunks
    KP = K + 1                  # candidate slots + sum-exp slot
    sz_g = bt * KP              # per-core allgather block (f32 elems)
    inv_tau = 1.0 / TAU

    nc = bacc.Bacc("TRN2", target_bir_lowering=False, debug=False,
                   num_devices=n_cores)

    tgtT_d = nc.dram_tensor("tgtT", [128, 2 * bt], BF16,
                            kind="ExternalInput")
    qT_d = nc.dram_tensor("qT", [128, 2 * QS], BF16, kind="ExternalInput")
    wt_d = nc.dram_tensor("wt", [128, 2 * C], BF16, kind="ExternalInput")
    labs_d = nc.dram_tensor("labs", [QS], I32, kind="ExternalInput")
    cnt_d = nc.dram_tensor("cnt", [C], F32, kind="ExternalInput")
    out_d = nc.dram_tensor("outv", [128, 2 * (BT // 128)], F32,
                           kind="ExternalOutput")
    red2_in = nc.dram_tensor("red2_in", [sz_g], F32)
    red2_out = nc.dram_tensor("red2_out", [n_cores * sz_g], F32,
                              addr_space="Shared" if n_cores > 4 else "Local")

    with tile.TileContext(nc) as tc:
        def _emit(ctx, gate_prev=False):
            sb = ctx.enter_context(tc.tile_pool(name="sb", bufs=1))

            # PE group-contiguity chain (see module docstring)
            _pe_prev = [None]

            def pe(bi):
                if _pe_prev[0] is not None:
                    add_dep_helper(bi.ins, _pe_prev[0].ins, sync=False,
                                   reason="PE group contiguity")
                _pe_prev[0] = bi
                return bi

            # chain gate for latency measurement: integer-add of
            # z = int(0*prev output) onto the raw bits (exact identity)
            if gate_prev:
                prevb = sb.tile([128, 1], F32, tag="prevb", name="prevb")
                nc.sync.dma_start(out=prevb[:], in_=out_d.ap()[:, 0:1])
                prevz = sb.tile([128, 1], F32, tag="prevz", name="prevz")
                nc.vector.tensor_scalar(prevz[:], prevb[:], 0.0, None,
                                        AL.mult)

            def gate(dst, src):
                # add exact-zero (0.0 * prev output) -- identity that
                # serializes this rep's inputs on the previous rep's out
                nc.vector.tensor_scalar(dst, src, prevz[:, 0:1], None,
                                        AL.add)

            # ---------- loads ----------
            # Each engine's DGE queue serializes its DMAs (and DMA-gated
            # cross-engine deps pay ~0.9us sem prop), so the 6.3MB of
            # input is split into kh / column halves and spread over the
            # three DMA-capable queues (SP / Pool / ACT) with
            # per-consumer deadlines; the ACT queue must drain before
            # the first exp:
            #   SP:     tgtT, qT kh0 halves, vp1 late half, qT sb2 kh0
            #   Pool:   qT kh1 of sb0/1, vp1 early half, vp2, wt, cnt
            #   scalar: vp0 halves, then the exps
            tgtT = sb.tile([128, 2, bt], BF16, tag="tgtT")
            nc.sync.dma_start(
                out=tgtT[:],
                in_=tgtT_d.ap().rearrange("p (kh t) -> p kh t", kh=2))
            if gate_prev:
                gate(tgtT[:], tgtT[:])
            qT = sb.tile([128, 2, QS], BF16, tag="qT")
            qTview = qT_d.ap().rearrange("p (kh q) -> p kh q", kh=2)
            H = SB // 2

            def qt_load(eng, s, kh, half):
                c0 = s * SB + half * H
                eng.dma_start(out=qT[:, kh:kh + 1, c0:c0 + H],
                              in_=qTview[:, kh:kh + 1, c0:c0 + H])

            # vp superblock buffers: whole-i32 broadcast puts the column
            # LABEL in the low u16 half (high halves are overwritten by
            # every ACT exp pass). One buffer per superblock: chunk-major
            # slot order gives reuse distance 3.
            vps = [sb.tile([128, SB], I32, tag=f"vp{s}", name=f"vp{s}")
                   for s in range(NSB)]

            def vp_load(eng, s, half):
                eng.dma_start(
                    out=vps[s][:, half * H:(half + 1) * H],
                    in_=labs_d.ap()[s * SB + half * H:
                                    s * SB + (half + 1) * H]
                    .unsqueeze(0).partition_broadcast(128))

            wt = sb.tile([128, 2, C], BF16, tag="wt")
            cnt_bc = sb.tile([128, C], F32, tag="cnt_bc")

            # scalar queue: vp0 only (after the framework's ACT table
            # load), then free for the exps
            vp_load(nc.scalar, 0, 0)
            vp_load(nc.scalar, 0, 1)
            # SP queue
            qt_load(nc.sync, 0, 0, 0)
            qt_load(nc.sync, 0, 0, 1)
            qt_load(nc.sync, 1, 0, 0)
            qt_load(nc.sync, 1, 0, 1)
            vp_load(nc.sync, 1, 1)
            qt_load(nc.sync, 2, 0, 0)
            qt_load(nc.sync, 2, 0, 1)
            nc.sync.dma_start(
                out=wt[:],
                in_=wt_d.ap().rearrange("p (kh c) -> p kh c", kh=2))
            if gate_prev:
                gate(wt[:], wt[:])
            # Pool queue
            qt_load(nc.gpsimd, 0, 1, 0)
            qt_load(nc.gpsimd, 0, 1, 1)
            qt_load(nc.gpsimd, 1, 1, 0)
            qt_load(nc.gpsimd, 1, 1, 1)
            vp_load(nc.gpsimd, 1, 0)
            vp_load(nc.gpsimd, 2, 0)
            vp_load(nc.gpsimd, 2, 1)
            qt_load(nc.gpsimd, 2, 1, 0)
            qt_load(nc.gpsimd, 2, 1, 1)
            nc.gpsimd.dma_start(
                out=cnt_bc[:],
                in_=cnt_d.ap().unsqueeze(0).partition_broadcast(128))
            ciota_i = sb.tile([128, C], I32, tag="ciota_i")
            nc.gpsimd.iota(ciota_i[:], pattern=[[1, C]], base=0,
                           channel_multiplier=0)
            ciota = sb.tile([128, C], F32, tag="ciota")
            nc.vector.tensor_copy(ciota[:], ciota_i[:])

            # ---------- compute ----------
            parts = sb.tile([128, tcn, NSB], F32, tag="parts")
            c24 = sb.tile([128, tcn, 8 * NSB], F32, tag="c24")
            se_packs = sb.tile([128, tcn, KP], F32, tag="se_packs")
            S_sb = sb.tile([128, tcn, C], F32, tag="S_sb")

            if stage >= 3:
                with ExitStack() as pctx:
                    psM = pctx.enter_context(
                        tc.tile_pool(name="psM", bufs=2, space="PSUM"))
                    # 12 sim slots; the order interleaves superblocks so
                    # every vp buffer has reuse distance >= 2 (ACT slot
                    # i+2 overwrites what DVE read at slot i) while
                    # superblock 2's data is not needed before slot 4,
                    # relaxing its DMA deadline
                    SLOT_ORDER = [(0, 0), (0, 1), (1, 0), (1, 1), (0, 2),
                                  (2, 0), (1, 2), (2, 1), (3, 0), (2, 2),
                                  (3, 1), (3, 2)]
                    for tci, s in SLOT_ORDER:
                        if True:
                            vp = vps[s]
                            ps = psM.tile([128, SB], F32, tag="mm",
                                          name="ps")
                            for piece in range(0, SB, 512):
                                n = min(512, SB - piece)
                                col = s * SB + piece
                                for kh in range(2):
                                    pe(nc.tensor.matmul(
                                        ps[:, piece:piece + n],
                                        lhsT=tgtT[:, kh, tci * 128:
                                                  (tci + 1) * 128],
                                        rhs=qT[:, kh, col:col + n],
                                        start=(kh == 0), stop=(kh == 1),
                                        skip_group_check=SKIP_GC))
                            vp_hi = vp[:].bitcast(BF16).rearrange(
                                "p (q two) -> p q two", two=2)[:, :, 1]
                            nc.scalar.activation(
                                vp_hi, ps[:], AF.Exp, scale=inv_tau,
                                accum_out=parts[:, tci, s:s + 1])
                            nc.vector.max(c24[:, tci, s * 8:s * 8 + 8],
                                          vp[:].bitcast(F32))

                    # S = tgtT^T @ W last: nothing reads S before the
                    # post-gather merge, so its matmuls go after the sim
                    # slots (PE idle) and its copy after the last exp
                    pS = psM.tile([128, SB], F32, tag="mm", name="pS")
                    for tci in range(tcn):
                        for kh in range(2):
                            pe(nc.tensor.matmul(
                                pS[:, tci * C:(tci + 1) * C],
                                lhsT=tgtT[:, kh,
                                          tci * 128:(tci + 1) * 128],
                                rhs=wt[:, kh], start=(kh == 0),
                                stop=(kh == 1), skip_group_check=SKIP_GC))
                    nc.scalar.activation(
                        S_sb[:].rearrange("p t c -> p (t c)"),
                        pS[:, 0:tcn * C], AF.Copy)

                # pack: top-5 of 24 per chunk + sum-exp slot
                nc.vector.reduce_sum(se_packs[:, :, K:K + 1], parts[:],
                                     axis=AX.X)
                vp8s = sb.tile([128, tcn, 8], F32, tag="vp8s")
                for tci in range(tcn):
                    nc.vector.max(vp8s[:, tci], c24[:, tci])
                nc.vector.tensor_copy(se_packs[:, :, 0:K],
                                      vp8s[:, :, 0:K])

            if stage >= 5:
                # ---------- AllGather of candidates + sum-exp ----------
                # the staging DMA, the collective and the readback all sit
                # on the Pool queue: in-order execution there replaces
                # three ~1us cross-engine DMA-sem propagation hops
                nc.gpsimd.dma_start(
                    out=red2_in.ap().rearrange("(t p k) -> p t k",
                                               p=128, k=KP),
                    in_=se_packs[:])
                if mock_cc:
                    for c in range(n_cores):
                        nc.gpsimd.dma_start(
                            out=red2_out.ap()[c * sz_g:(c + 1) * sz_g],
                            in_=red2_in.ap())
                else:
                    nc.gpsimd.collective_compute(
                        "AllGather", AL.bypass,
                        replica_groups=[list(range(n_cores))],
                        ins=[red2_in.ap().opt()],
                        outs=[red2_out.ap().opt()])

            if stage >= 6:
                # ---------- final merge / vote / loss (redundant) ----------
                cands6 = sb.tile([128, n_cores * tcn, KP], F32,
                                 tag="cands6")
                nc.gpsimd.dma_start(
                    out=cands6[:],
                    in_=red2_out.ap().rearrange("(ct p k) -> p ct k",
                                                p=128, k=KP))
                ctv = cands6[:].rearrange("p (c t) k -> p t c k",
                                          c=n_cores)
                se_tot = sb.tile([128, tcn, 1], F32, tag="se_tot")
                nc.vector.reduce_sum(se_tot[:], ctv[:, :, :, K],
                                     axis=AX.X)

                g40 = sb.tile([128, tcn, 8], F32, tag="g40")
                for tci in range(tcn):
                    nc.vector.max(g40[:, tci], ctv[:, tci, :, 0:K])
                # labels ride the low u16 halves of the packed values
                lab20i = sb.tile([128, tcn, K], I32, tag="lab20i")
                nc.vector.tensor_scalar(lab20i[:],
                                        g40[:, :, 0:K].bitcast(I32),
                                        65535, None, AL.bitwise_and)
                lab20 = sb.tile([128, tcn, K], F32, tag="lab20")
                nc.vector.tensor_copy(lab20[:], lab20i[:])
                # all-pairs vote in two wide ops
                cnt20 = sb.tile([128, tcn, K], F32, tag="cnt20")
                eq_scr = sb.tile([128, tcn, K, K], F32, tag="eq_scr")
                nc.vector.tensor_tensor(
                    out=eq_scr[:],
                    in0=lab20[:].unsqueeze(3).broadcast_to(
                        [128, tcn, K, K]),
                    in1=lab20[:].unsqueeze(2).broadcast_to(
                        [128, tcn, K, K]),
                    op=AL.is_equal)
                nc.vector.reduce_sum(cnt20[:].unsqueeze(3), eq_scr[:],
                                     axis=AX.X)
                score = sb.tile([128, tcn, K], F32, tag="score")
                nc.vector.scalar_tensor_tensor(
                    out=score[:], in0=cnt20[:], scalar=1024.0,
                    in1=lab20[:], op0=AL.mult, op1=AL.subtract)
                nc.vector.tensor_scalar(score[:], score[:], 1023.0, None,
                                        AL.add)
                best = sb.tile([128, tcn, 1], F32, tag="best")
                nc.vector.reduce_max(best[:], score[:], axis=AX.X)
                besti = sb.tile([128, tcn], I32, tag="besti")
                nc.vector.tensor_copy(besti[:], best[:, :, 0])
                encb = sb.tile([128, tcn], I32, tag="encb")
                nc.vector.tensor_scalar(encb[:], besti[:], 1023, None,
                                        AL.bitwise_and)
                pseudo = sb.tile([128, tcn], F32, tag="pseudo")
                nc.vector.tensor_scalar(pseudo[:], encb[:], -1, 1023,
                                        AL.mult, AL.add)
                spos = sb.tile([128, tcn], F32, tag="spos")
                cntp = sb.tile([128, tcn], F32, tag="cntp")
                junk = sb.tile([128, C], F32, tag="junk")
                for tci in range(tcn):
                    nc.vector.scalar_tensor_tensor(
                        out=junk[:], in0=ciota[:],
                        scalar=pseudo[:, tci:tci + 1], in1=S_sb[:, tci],
                        op0=AL.is_equal, op1=AL.mult,
                        accum_out=spos[:, tci:tci + 1])
                    nc.vector.scalar_tensor_tensor(
                        out=junk[:], in0=ciota[:],
                        scalar=pseudo[:, tci:tci + 1], in1=cnt_bc[:],
                        op0=AL.is_equal, op1=AL.mult,
                        accum_out=cntp[:, tci:tci + 1])
                rc = sb.tile([128, tcn], F32, tag="rc")
                nc.vector.reciprocal(rc[:], cntp[:])
                # ship (positive-mean, sum-exp) per sample; the host
                # gather applies log, the warm-up coefficient and the
                # batch mean (keeping Ln off the device keeps the Exp
                # ACT table resident across reps)
                ps_pack = sb.tile([128, 2, tcn], F32, tag="ps_pack")
                nc.vector.scalar_tensor_tensor(
                    out=ps_pack[:, 0], in0=spos[:], scalar=inv_tau,
                    in1=rc[:], op0=AL.mult, op1=AL.mult)
                nc.vector.tensor_copy(ps_pack[:, 1], se_tot[:, :, 0])
                nc.sync.dma_start(
                    out=out_d.ap().rearrange("p (two t) -> p two t",
                                             two=2),
                    in_=ps_pack[:])
            if stage < 6:
                dres = sb.tile([128, 2, tcn], F32, tag="dres")
                nc.vector.memset(dres[:], 1.0)
                nc.sync.dma_start(
                    out=out_d.ap().rearrange("p (two t) -> p two t",
                                             two=2),
                    in_=dres[:])

        for _rep in range(n_reps):
            with ExitStack() as ctx:
                _emit(ctx, gate_prev=(chain and _rep > 0))

    nc.compile()
    return nc


def make_in_maps(features, source_labels, it, queue, queue_labels,
                 n_cores=NCORES, qsp=QSP):
    """Host-side sharding glue: substitute enqueued rows, normalize,
    shard, build device layouts + global class sums / counts."""
    features = np.asarray(features, dtype=np.float32)
    queue = np.asarray(queue, dtype=np.float32)
    src_lab = np.asarray(source_labels).astype(np.int64)
    q_lab = np.asarray(queue_labels).astype(np.int64)
    bs = src_lab.shape[0]
    qs = queue.shape[0] // n_cores

    src = features[:bs]
    tgt = np.ascontiguousarray(features[bs:])
    newq = queue.copy()
    newq[:bs] = src
    newl = q_lab.copy()
    newl[:bs] = src_lab

    # row-wise L2 normalize (matches F.normalize eps)
    qn = newq / np.maximum(
        np.linalg.norm(newq, axis=1, keepdims=True), 1e-12)
    tgtn = tgt / np.maximum(
        np.linalg.norm(tgt, axis=1, keepdims=True), 1e-12)
    # [p, kh*bt]: tgtT[p, kh, t] = tgtn[t, kh*128+p]
    tgtT = np.ascontiguousarray(
        tgtn.T.reshape(2, 128, bs).transpose(1, 0, 2).reshape(128, -1)
    ).astype(NP_BF16)
    cnt_glob = np.bincount(newl, minlength=C)[:C].astype(np.float32)
    # global class sums of the normalized queue (label-histogram glue)
    W = np.zeros((C + 1, D), np.float32)
    np.add.at(W, newl, qn)
    wt = np.ascontiguousarray(
        W[:C].T.reshape(2, 128, C).transpose(1, 0, 2).reshape(128, -1)
    ).astype(NP_BF16)

    in_maps = []
    for c in range(n_cores):
        shard = qn[c * qs:(c + 1) * qs]
        qT = np.ascontiguousarray(
            shard.T.reshape(2, 128, qs).transpose(1, 0, 2).reshape(
                128, -1)).astype(NP_BF16)
        labs = np.ascontiguousarray(
            newl[c * qs:(c + 1) * qs].astype(np.int32))
        in_maps.append({
            "tgtT": tgtT,
            "qT": qT,
            "wt": wt,
            "labs": labs,
            "cnt": cnt_glob,
        })
    return in_maps


_CACHED = {}


def _get_program():
    key = (NCORES, QSP, BT)
    if key not in _CACHED:
        _CACHED[key] = build_program(*key)
    return _CACHED[key]


def kernel(**inputs):
    nc = _get_program()
    in_maps = make_in_maps(inputs["features"], inputs["source_labels"],
                           inputs["it"], inputs["queue"],
                           inputs["queue_labels"])
    res = run_bass_kernel_spmd(nc, in_maps, core_ids=list(range(NCORES)))
    ps = np.asarray(res.results[0]["outv"], np.float64)
    tcn = BT // 128
    mp = ps[:, 0:tcn]
    se = ps[:, tcn:2 * tcn]
    per_sample = np.log(se) - mp
    coeff = COEFF if float(np.asarray(inputs["it"])) > WARM_UP else 0.0
    out = np.float32(coeff * per_sample.mean())
    return out
